# revision 6
# baseline (speedup 1.0000x reference)
"""LSTM-encoder (VAE head) Trainium kernel — collective-free replication.

Why replication: the recurrence h_t -> h_{t+1} is strictly sequential, and a
per-step 8-core AllGather costs ~3 ms through this runtime (512 collectives
dominated the previous version at ~1.4-2.0 s total). Instead, every core
redundantly computes the FULL model; core 0's outputs are returned. Per-step
PE work is B x (D+H) x 4H MACs = 671M -> 40960 PE cycles (~17 us at 2.4 GHz),
so T=512 steps ~ 9 ms — two orders of magnitude below the collective path.

Layout per step (batch-major): gates [B=128(part), 4096] live in 8 PSUM
groups of 512 columns. Group j holds the 4 gate blocks of hidden chunk j in
order [-g | i | f | o] (host negates the g-block weight columns so a single
Exp(scale=-1) read yields [e^g | e^-i | e^-f | e^-o]). Then per chunk:
    softplus(g) = Ln(e^g + 1)          (ACT)
    sigmoid(x)  = 1/(1 + e^-x)         (DVE add + reciprocal)
    c = f*c + i*softplus(g)            (DVE/Pool)
    h = o * softplus(c)                (ACT Exp/Ln + DVE mul)
    hT_j = h_j.T via PE transpose, ACT-copied to SBUF for the next step's
    stationary operand.

Transposes are staggered ~2 PSUM groups behind their producing chain (the
last two chunks spill into the next step's first matmul group) so the PE
never waits on the elementwise chain except at the structural step boundary.

mm_dt: dtype of TensorE-facing tensors (bfloat16 default).
"""

import numpy as np

import concourse.bass as bass
import concourse.mybir as mybir
import concourse.tile as tile
from concourse.masks import make_identity

AF = mybir.ActivationFunctionType
F32 = mybir.dt.float32
BF16 = mybir.dt.bfloat16

B, D, H, Z = 128, 256, 1024, 128
NCORES = 8
KH = H // 128              # 8 hidden chunks == 8 PSUM gate groups per step
KD = D // 128              # 2 x chunks
GC = 512                   # gate columns per group: [g|i|f|o] x 128
HS = 128                   # hidden units per chunk


# Engine-compute ISA structs carry a limited number of sync-wait slots.
# Tile's sem assigner can emit more. Spill the excess onto standalone
# EventSemaphore wait instructions inserted just before the offender in the
# same engine stream — identical semantics, a few ns of dispatch.
_SPILL_TYPES = (
    "InstMatmult",
    "InstTensorTensor",
    "InstActivation",
    "InstTensorCopy",
    "InstTensorScalarPtr",
    "InstReciprocal",
    "InstMemset",
    "InstNoOp",
    "InstLdweights",
    "InstCopyPredicated",
    "InstTensorScalarAffineSelect",
    "InstCollectiveCompute",
    "InstEventSemaphore",
    "InstDrain",
    "InstDMACopy",
)

_WAIT_LIMITS = {}


def _spill_excess_waits(nc, limit=1):
    f = nc.m.functions[0]
    n_spilled = 0
    for bb in f.blocks:
        out = []
        for inst in bb.instructions:
            si = inst.sync_info
            waits = list(si.on_wait) if si and si.on_wait else []
            tname = type(inst).__name__
            limit_t = _WAIT_LIMITS.get(tname, limit)
            if tname in _SPILL_TYPES and len(waits) > limit_t:
                keep = waits[len(waits) - limit_t :]
                for w in waits[: len(waits) - limit_t]:
                    es = mybir.InstEventSemaphore(
                        name=f"WSPILL-{n_spilled}-{inst.name}",
                        engine=inst.engine,
                        ins=[],
                        outs=[],
                        sync_info=mybir.SyncInfo(on_wait=[w], on_update=[]),
                    )
                    out.append(es)
                    n_spilled += 1
                si.on_wait = keep
            out.append(inst)
        bb.instructions = out
    return n_spilled


def build_nc(
    T: int,
    mm_dt=BF16,
    T_data=None,
    has_bias=True,
    n_dev=NCORES,
    fp8_dr=False,
    w_scale=1.0,
):
    """fp8_dr: use float8e4 DoubleRow matmuls for the recurrence (2x PE rate;
    K=256 per matmul via paired chunks). The head matmul stays bf16 via a
    separate bf16 copy of the final hidden state. w_scale: power-of-2 factor
    the host pre-multiplied into W/U/b to dodge the fp8 subnormal floor —
    undone for free via the Exp activation's scale operand."""
    if T_data is None:
        T_data = T
    if fp8_dr:
        mm_dt = mybir.dt.float8e4
    head_dt = BF16
    nc = bass.Bass(
        "TRN2", target_bir_lowering=False, debug=False, num_devices=n_dev
    )

    xT_d = nc.dram_tensor("xT", [T_data, D, B], mm_dt, kind="ExternalInput")
    U_d = nc.dram_tensor("Usl", [KH, 128, KH * GC], mm_dt, kind="ExternalInput")
    W_d = nc.dram_tensor("Wsl", [KD, 128, KH * GC], mm_dt, kind="ExternalInput")
    b_d = nc.dram_tensor("bsl", [1, KH * GC], mm_dt, kind="ExternalInput")
    Wm_d = nc.dram_tensor("Wm", [KH, 128, Z], head_dt, kind="ExternalInput")
    Wv_d = nc.dram_tensor("Wv", [KH, 128, Z], head_dt, kind="ExternalInput")
    bm_d = nc.dram_tensor("bm", [Z, 1], F32, kind="ExternalInput")
    bv_d = nc.dram_tensor("bv", [Z, 1], F32, kind="ExternalInput")
    bvh_d = nc.dram_tensor("bvh", [Z, 1], F32, kind="ExternalInput")
    epsT_d = nc.dram_tensor("epsT", [Z, B], F32, kind="ExternalInput")

    muT_d = nc.dram_tensor("muT", [Z, B], F32, kind="ExternalOutput")
    lvT_d = nc.dram_tensor("logvarT", [Z, B], F32, kind="ExternalOutput")
    zT_d = nc.dram_tensor("zT", [Z, B], F32, kind="ExternalOutput")

    with tile.TileContext(nc) as tc:
        with (
            tc.tile_pool(name="const", bufs=1) as cpool,
            tc.tile_pool(name="xt", bufs=4) as xt_pool,
            tc.tile_pool(name="hTg", bufs=2) as hT_pool,
            tc.tile_pool(name="gps", bufs=6, space="PSUM") as gps_pool,
            tc.tile_pool(name="trps", bufs=2, space="PSUM") as trps_pool,
            tc.tile_pool(name="headps", bufs=1, space="PSUM") as head_pool,
            tc.tile_pool(name="act", bufs=4) as apool,
        ):
            # ---- constants / persistent state ----
            U_sb = cpool.tile([128, KH, KH * GC], mm_dt, tag="U")
            nc.sync.dma_start(out=U_sb[:], in_=U_d.ap().rearrange("c p g -> p c g"))
            W_sb = cpool.tile([128, KD, KH * GC], mm_dt, tag="W")
            nc.sync.dma_start(out=W_sb[:], in_=W_d.ap().rearrange("c p g -> p c g"))
            if has_bias:
                b_sb = cpool.tile([1, KH * GC], mm_dt, tag="b")
                nc.sync.dma_start(out=b_sb[:], in_=b_d.ap())
                ones_sb = cpool.tile([1, B], mm_dt, tag="ones")
                nc.vector.memset(ones_sb[:], 1.0)
            ident = cpool.tile([128, 128], BF16, tag="ident")
            make_identity(nc, ident[:])
            c_sb = cpool.tile([128, H], F32, tag="c")
            nc.vector.memset(c_sb[:], 0.0)

            Wm_sb = cpool.tile([128, KH, Z], head_dt, tag="Wm")
            nc.sync.dma_start(out=Wm_sb[:], in_=Wm_d.ap().rearrange("c p z -> p c z"))
            Wv_sb = cpool.tile([128, KH, Z], head_dt, tag="Wv")
            nc.sync.dma_start(out=Wv_sb[:], in_=Wv_d.ap().rearrange("c p z -> p c z"))
            bm_sb = cpool.tile([Z, 1], F32, tag="bm")
            nc.sync.dma_start(out=bm_sb[:], in_=bm_d.ap())
            bv_sb = cpool.tile([Z, 1], F32, tag="bv")
            nc.sync.dma_start(out=bv_sb[:], in_=bv_d.ap())
            bvh_sb = cpool.tile([Z, 1], F32, tag="bvh")
            nc.sync.dma_start(out=bvh_sb[:], in_=bvh_d.ap())
            epsT_sb = cpool.tile([Z, B], F32, tag="epsT")
            nc.sync.dma_start(out=epsT_sb[:], in_=epsT_d.ap())

            # previous-step transposed hidden state (zeros at t=0)
            hT_prev = hT_pool.tile([128, KH, B], mm_dt, tag="hT")
            nc.vector.memset(hT_prev[:], 0.0)

            xT_r = xT_d.ap().rearrange("t (c p) b -> t p c b", p=128)

            # warm-up transpose: folds the identity-matrix (gpsimd) dep into
            # the PE clock so per-step transposes carry a single wait
            tr_warm = trps_pool.tile([128, B], BF16, tag="trps")
            nc.tensor.transpose(tr_warm[:], ident[:], ident[:])

            # pending transposes: (step, group, mm_pos) -> list of (chunk,
            # h_tile, hT_dest_tile). Chunk k of step t is transposed at:
            #   k <= 5 -> (t, k+2, 0);  k == 6 -> (t+1, 0, 4);
            #   k == 7 -> (t+1, 0, 6)   (after the consumers' slack window)
            pending = {}

            def emit_tr(chunk, h_tile, hT_dest):
                tr_ps = trps_pool.tile([128, B], BF16, tag="trps")
                nc.tensor.transpose(tr_ps[:], h_tile[:], ident[:])
                nc.scalar.copy(hT_dest[:, chunk, :], tr_ps[:])

            for t in range(T):
                xt = xt_pool.tile([128, KD, B], mm_dt, tag="xt")
                nc.sync.dma_start(out=xt[:], in_=xT_r[t % T_data])

                hT_next = hT_pool.tile([128, KH, B], mm_dt, tag="hT")

                for g in range(KH):
                    cols = slice(g * GC, (g + 1) * GC)
                    g_ps = gps_pool.tile([128, GC], F32, tag="gps")

                    # matmul sequence for this gate group; pending transposes
                    # are interleaved at their scheduled positions
                    n_mm = (1 if has_bias else 0) + KD + KH
                    pos = 0

                    def flush_pending(pos, g=g):
                        for item in pending.pop((t, g, pos), []):
                            emit_tr(*item)

                    flush_pending(pos)
                    if has_bias:
                        nc.tensor.matmul(
                            g_ps[:], ones_sb[:], b_sb[:, cols],
                            start=True, stop=False,
                        )
                        pos += 1
                    for c in range(KD):
                        flush_pending(pos)
                        nc.tensor.matmul(
                            g_ps[:], xt[:, c, :], W_sb[:, c, cols],
                            start=(pos == 0), stop=False,
                        )
                        pos += 1
                    for k in range(KH):
                        flush_pending(pos)
                        nc.tensor.matmul(
                            g_ps[:], hT_prev[:, k, :], U_sb[:, k, cols],
                            start=False, stop=(k == KH - 1),
                        )
                        pos += 1

                    # ---- elementwise chain for hidden chunk g ----
                    cs = slice(g * HS, (g + 1) * HS)
                    e_all = apool.tile([128, GC], BF16, tag="eall")
                    nc.scalar.activation(e_all[:], g_ps[:], AF.Exp, scale=-1.0)
                    spg = apool.tile([128, HS], F32, tag="spg")
                    nc.scalar.activation(spg[:], e_all[:, 0:HS], AF.Ln, bias=1.0)
                    d_ifo = apool.tile([128, 3 * HS], BF16, tag="difo")
                    nc.vector.tensor_scalar_add(d_ifo[:], e_all[:, HS:GC], 1.0)
                    sig = apool.tile([128, 3 * HS], BF16, tag="sig")
                    with nc.allow_low_precision("bf16 sigmoid is fine"):
                        nc.vector.reciprocal(sig[:], d_ifo[:])

                    t_ig = apool.tile([128, HS], F32, tag="tig")
                    nc.vector.tensor_mul(t_ig[:], sig[:, 0:HS], spg[:])
                    t_fc = apool.tile([128, HS], F32, tag="tfc")
                    nc.gpsimd.tensor_mul(t_fc[:], sig[:, HS : 2 * HS], c_sb[:, cs])
                    nc.vector.tensor_add(c_sb[:, cs], t_ig[:], t_fc[:])

                    ec = apool.tile([128, HS], BF16, tag="ec")
                    nc.scalar.activation(ec[:], c_sb[:, cs], AF.Exp)
                    spc = apool.tile([128, HS], F32, tag="spc")
                    nc.scalar.activation(spc[:], ec[:], AF.Ln, bias=1.0)
                    h_sb = apool.tile([128, HS], BF16, tag="h")
                    nc.vector.tensor_mul(h_sb[:], sig[:, 2 * HS : 3 * HS], spc[:])

                    # schedule this chunk's transpose into the PE stream
                    if g <= 5:
                        key = (t, g + 2, 0)
                    elif g == 6:
                        key = (t + 1, 0, 4)
                    else:
                        key = (t + 1, 0, 6)
                    pending.setdefault(key, []).append((g, h_sb, hT_next))

                hT_prev = hT_next

            # flush transposes scheduled past the last step
            for key in sorted(pending.keys()):
                for item in pending.pop(key):
                    emit_tr(*item)

            # ---- VAE head ----
            mu_ps = head_pool.tile([Z, B], F32, tag="head_mu")
            lv_ps = head_pool.tile([Z, B], F32, tag="head_lv")
            for c in range(KH):
                nc.tensor.matmul(
                    mu_ps[:], Wm_sb[:, c, :], hT_prev[:, c, :],
                    start=(c == 0), stop=(c == KH - 1),
                )
            for c in range(KH):
                nc.tensor.matmul(
                    lv_ps[:], Wv_sb[:, c, :], hT_prev[:, c, :],
                    start=(c == 0), stop=(c == KH - 1),
                )

            mu_sb = apool.tile([Z, B], F32, tag="mu")
            nc.scalar.activation(mu_sb[:], mu_ps[:], AF.Identity, bias=bm_sb[:])
            lv_sb = apool.tile([Z, B], F32, tag="lv")
            nc.scalar.activation(lv_sb[:], lv_ps[:], AF.Identity, bias=bv_sb[:])
            e_sb = apool.tile([Z, B], F32, tag="e")
            nc.scalar.activation(
                e_sb[:], lv_ps[:], AF.Exp, bias=bvh_sb[:], scale=0.5
            )
            ez = apool.tile([Z, B], F32, tag="ez")
            nc.vector.tensor_mul(ez[:], e_sb[:], epsT_sb[:])
            z_sb = apool.tile([Z, B], F32, tag="z")
            nc.vector.tensor_add(z_sb[:], mu_sb[:], ez[:])

            nc.sync.dma_start(out=muT_d.ap(), in_=mu_sb[:])
            nc.sync.dma_start(out=lvT_d.ap(), in_=lv_sb[:])
            nc.sync.dma_start(out=zT_d.ap(), in_=z_sb[:])

    _spill_excess_waits(nc)
    return nc


def make_in_maps(x, W, U, b, Wm, bm, Wv, bv, eps, np_mm_dtype=None):
    import ml_dtypes
    if np_mm_dtype is None:
        np_mm_dtype = ml_dtypes.bfloat16
    """Host-side pre-processing: transpose x, permute weight columns into
    per-chunk [g|i|f|o] blocks (g negated). Same map for every core."""
    T = x.shape[1]
    xT = np.ascontiguousarray(x.transpose(1, 2, 0)).astype(np_mm_dtype)  # [T,D,B]
    epsT = np.ascontiguousarray(eps.T).astype(np.float32)  # [Z, B]
    bm_c = np.ascontiguousarray(bm.reshape(Z, 1)).astype(np.float32)
    bv_c = np.ascontiguousarray(bv.reshape(Z, 1)).astype(np.float32)
    bvh_c = np.ascontiguousarray(0.5 * bv.reshape(Z, 1)).astype(np.float32)
    Wm_r = np.ascontiguousarray(Wm.reshape(KH, 128, Z)).astype(np_mm_dtype)
    Wv_r = np.ascontiguousarray(Wv.reshape(KH, 128, Z)).astype(np_mm_dtype)

    # gate order per chunk [g | i | f | o]; Keras kernel order is i,f,g,o
    cols = np.concatenate(
        [
            np.concatenate(
                [
                    np.arange(2 * H + j * HS, 2 * H + (j + 1) * HS),  # g
                    np.arange(0 * H + j * HS, 0 * H + (j + 1) * HS),  # i
                    np.arange(1 * H + j * HS, 1 * H + (j + 1) * HS),  # f
                    np.arange(3 * H + j * HS, 3 * H + (j + 1) * HS),  # o
                ]
            )
            for j in range(KH)
        ]
    )
    # negate the g-block columns so one Exp(scale=-1) serves all gates
    neg = np.ones((KH * GC,), np.float32)
    for j in range(KH):
        neg[j * GC : j * GC + HS] = -1.0
    Usl = np.ascontiguousarray(U[:, cols] * neg).reshape(KH, 128, KH * GC).astype(
        np_mm_dtype
    )
    Wsl = np.ascontiguousarray(W[:, cols] * neg).reshape(KD, 128, KH * GC).astype(
        np_mm_dtype
    )
    bsl = (b[cols] * neg).reshape(1, KH * GC).astype(np_mm_dtype)
    m = {
        "xT": xT,
        "Usl": Usl,
        "Wsl": Wsl,
        "bsl": bsl,
        "Wm": Wm_r,
        "Wv": Wv_r,
        "bm": bm_c,
        "bv": bv_c,
        "bvh": bvh_c,
        "epsT": epsT,
    }
    return [m for _ in range(NCORES)]


def postprocess(core0_out):
    mu = np.ascontiguousarray(core0_out["muT"].T).astype(np.float32)
    logvar = np.ascontiguousarray(core0_out["logvarT"].T).astype(np.float32)
    z = np.ascontiguousarray(core0_out["zT"].T).astype(np.float32)
    return mu, logvar, z


# ----------------------------------------------------------------------------
# Harness entry point: full (unsharded) inputs -> full outputs.
# ----------------------------------------------------------------------------
_NC_CACHE = {}


def kernel(x, W, U, b, Wm, bm, Wv, bv, eps):
    import time as _time

    from concourse.bass_utils import run_bass_kernel_spmd

    x = np.asarray(x, dtype=np.float32)
    W = np.asarray(W, dtype=np.float32)
    U = np.asarray(U, dtype=np.float32)
    b = np.asarray(b, dtype=np.float32)
    Wm = np.asarray(Wm, dtype=np.float32)
    bm = np.asarray(bm, dtype=np.float32)
    Wv = np.asarray(Wv, dtype=np.float32)
    bv = np.asarray(bv, dtype=np.float32)
    eps = np.asarray(eps, dtype=np.float32)

    T = x.shape[1]
    has_bias = bool(np.any(b != 0.0))
    key = (T, has_bias)
    if key not in _NC_CACHE:
        _NC_CACHE[key] = build_nc(T, has_bias=has_bias)
    nc = _NC_CACHE[key]
    in_maps = make_in_maps(x, W, U, b, Wm, bm, Wv, bv, eps)
    last = None
    for _attempt in range(3):
        try:
            res = run_bass_kernel_spmd(nc, in_maps, core_ids=list(range(NCORES)))
            return postprocess(res.results[0])
        except Exception as e:  # transient device hiccups: retry
            last = e
            _time.sleep(2.0)
    raise last


# revision 18
# speedup vs baseline: 11.8868x; 11.8868x over previous
"""LSTM-encoder (VAE head) Trainium kernel — instruction-count-minimized
hidden-split.

The axon execution path charges a large, roughly flat cost PER INSTRUCTION
(measured: matmul ~72us regardless of shape, activation ~110us, DVE op
~35-40us, DMA ~38us, AllGather ~167us; engines within a core serialize,
the 8 cores run in parallel). Optimization therefore means minimizing the
per-step instruction count per core, using the widest ops possible.

Distribution: hidden-split across 8 cores. Core k computes gates for its
128 hidden units (512 gate columns, order [g|i|f|o], g negated) for the
full batch B=128. Per step per core:

  matmuls (PSUM [B,512]): bf16 10 (2 x-chunks + 8 h-chunks) or fp8
    DoubleRow 5 (1 x-pair + 4 h-pairs, K=256 each)
  chain: e=Exp(-gates) | spg=Ln(e_g+1) | d=e_ifo+1 | sig=1/d
         [tig|tfc] = sig[:,0:256] * [spg|c] (one DVE op via adjacency)
         c = tig+tfc | ec=Exp(c) | spc=Ln(ec+1) | h = sig_o*spc
  exchange (no PE transpose): dma h [B,128] -> DRAM, AllGather ->
    [(core b), h] = [8B, 128], one transposing gather dma -> hT [h,c,b]
  x is DMA'd in blocks of XBLK steps.

Head (all cores redundantly): muT = Wm.T @ hT + bm, logvarT likewise,
zT = muT + epsT * exp(0.5*logvarT). Host transposes back.
"""

import numpy as np

import concourse.bass as bass
import concourse.mybir as mybir
import concourse.tile as tile

AF = mybir.ActivationFunctionType
F32 = mybir.dt.float32
BF16 = mybir.dt.bfloat16

B, D, H, Z = 128, 256, 1024, 128
NCORES = 8
HS = H // NCORES          # 128 hidden units per core
GC = 4 * HS               # 512 gate columns per core
KH = H // 128             # 8 h chunks
KD = D // 128             # 2 x chunks
XBLK = 16                 # steps of x per input DMA


_SPILL_TYPES = (
    "InstMatmult",
    "InstTensorTensor",
    "InstActivation",
    "InstTensorCopy",
    "InstTensorScalarPtr",
    "InstReciprocal",
    "InstMemset",
    "InstNoOp",
    "InstLdweights",
    "InstCopyPredicated",
    "InstTensorScalarAffineSelect",
    "InstCollectiveCompute",
    "InstEventSemaphore",
    "InstDrain",
    "InstDMACopy",
)

_WAIT_LIMITS = {}


def _spill_excess_waits(nc, limit=1):
    f = nc.m.functions[0]
    n_spilled = 0
    for bb in f.blocks:
        out = []
        for inst in bb.instructions:
            si = inst.sync_info
            waits = list(si.on_wait) if si and si.on_wait else []
            tname = type(inst).__name__
            limit_t = _WAIT_LIMITS.get(tname, limit)
            if tname in _SPILL_TYPES and len(waits) > limit_t:
                keep = waits[len(waits) - limit_t :]
                for w in waits[: len(waits) - limit_t]:
                    es = mybir.InstEventSemaphore(
                        name=f"WSPILL-{n_spilled}-{inst.name}",
                        engine=inst.engine,
                        ins=[],
                        outs=[],
                        sync_info=mybir.SyncInfo(on_wait=[w], on_update=[]),
                    )
                    out.append(es)
                    n_spilled += 1
                si.on_wait = keep
            out.append(inst)
        bb.instructions = out
    return n_spilled


def build_nc(
    T: int,
    mm_dt=BF16,
    T_data=None,
    has_bias=True,
    n_dev=NCORES,
    fp8_dr=False,
    w_scale=1.0,
    strip="none",
):
    if T_data is None:
        T_data = T
    T_data = ((T_data + XBLK - 1) // XBLK) * XBLK  # padded by make_in_maps
    if fp8_dr:
        mm_dt = mybir.dt.float8e4
    head_dt = BF16
    nc = bass.Bass(
        "TRN2", target_bir_lowering=False, debug=False, num_devices=n_dev
    )

    xT_d = nc.dram_tensor("xT", [T_data, D, B], mm_dt, kind="ExternalInput")
    U_d = nc.dram_tensor("Usl", [KH, 128, GC], mm_dt, kind="ExternalInput")
    W_d = nc.dram_tensor("Wsl", [KD, 128, GC], mm_dt, kind="ExternalInput")
    b_d = nc.dram_tensor("bsl", [1, GC], mm_dt, kind="ExternalInput")
    Wm_d = nc.dram_tensor("Wm", [KH, 128, Z], head_dt, kind="ExternalInput")
    Wv_d = nc.dram_tensor("Wv", [KH, 128, Z], head_dt, kind="ExternalInput")
    bm_d = nc.dram_tensor("bm", [Z, 1], F32, kind="ExternalInput")
    bv_d = nc.dram_tensor("bv", [Z, 1], F32, kind="ExternalInput")
    bvh_d = nc.dram_tensor("bvh", [Z, 1], F32, kind="ExternalInput")
    epsT_d = nc.dram_tensor("epsT", [Z, B], F32, kind="ExternalInput")

    muT_d = nc.dram_tensor("muT", [Z, B], F32, kind="ExternalOutput")
    lvT_d = nc.dram_tensor("logvarT", [Z, B], F32, kind="ExternalOutput")
    zT_d = nc.dram_tensor("zT", [Z, B], F32, kind="ExternalOutput")

    with tile.TileContext(nc) as tc:
        with (
            tc.tile_pool(name="const", bufs=1) as cpool,
            tc.tile_pool(name="xt", bufs=2) as xt_pool,
            tc.tile_pool(name="hTg", bufs=2) as hT_pool,
            tc.tile_pool(name="gps", bufs=3, space="PSUM") as gps_pool,
            tc.tile_pool(name="act", bufs=3) as apool,
            tc.tile_pool(name="ccd", bufs=2, space="DRAM") as dpool,
        ):
            # ---- constants / persistent state ----
            U_sb = cpool.tile([128, KH, GC], mm_dt, tag="U")
            nc.sync.dma_start(out=U_sb[:], in_=U_d.ap().rearrange("c p g -> p c g"))
            W_sb = cpool.tile([128, KD, GC], mm_dt, tag="W")
            nc.sync.dma_start(out=W_sb[:], in_=W_d.ap().rearrange("c p g -> p c g"))
            if has_bias:
                b_sb = cpool.tile([1, GC], mm_dt, tag="b")
                nc.sync.dma_start(out=b_sb[:], in_=b_d.ap())
                ones_sb = cpool.tile([1, B], mm_dt, tag="ones")
                nc.vector.memset(ones_sb[:], 1.0)

            # spg | c adjacency tile: [:, 0:HS] = softplus(g) scratch,
            # [:, HS:2HS] = persistent cell state c
            spgc = cpool.tile([128, 2 * HS], F32, tag="spgc")
            nc.vector.memset(spgc[:], 0.0)

            Wm_sb = cpool.tile([128, KH, Z], head_dt, tag="Wm")
            nc.sync.dma_start(out=Wm_sb[:], in_=Wm_d.ap().rearrange("c p z -> p c z"))
            Wv_sb = cpool.tile([128, KH, Z], head_dt, tag="Wv")
            nc.sync.dma_start(out=Wv_sb[:], in_=Wv_d.ap().rearrange("c p z -> p c z"))
            bm_sb = cpool.tile([Z, 1], F32, tag="bm")
            nc.sync.dma_start(out=bm_sb[:], in_=bm_d.ap())
            bv_sb = cpool.tile([Z, 1], F32, tag="bv")
            nc.sync.dma_start(out=bv_sb[:], in_=bv_d.ap())
            bvh_sb = cpool.tile([Z, 1], F32, tag="bvh")
            nc.sync.dma_start(out=bvh_sb[:], in_=bvh_d.ap())
            epsT_sb = cpool.tile([Z, B], F32, tag="epsT")
            nc.sync.dma_start(out=epsT_sb[:], in_=epsT_d.ap())

            # initial gathered hidden state (zeros)
            hT_prev = hT_pool.tile([128, KH, B], mm_dt, tag="hT")
            nc.vector.memset(hT_prev[:], 0.0)

            xT_r = xT_d.ap().rearrange("(tb ts) (c p) b -> tb p ts c b",
                                       p=128, ts=XBLK)

            xt_blk = None
            for t in range(T):
                ts = t % XBLK
                if ts == 0:
                    xt_blk = xt_pool.tile([128, XBLK, KD, B], mm_dt, tag="xt")
                    nc.sync.dma_start(
                        out=xt_blk[:], in_=xT_r[(t // XBLK) % (T_data // XBLK)]
                    )

                g_ps = gps_pool.tile([128, GC], F32, tag="gps")
                if fp8_dr:
                    nmm = 1 + KH // 2 + (1 if has_bias else 0)
                    pos = 0
                    if has_bias:
                        nc.tensor.matmul(g_ps[:], ones_sb[:], b_sb[:],
                                         start=True, stop=False)
                        pos += 1
                    nc.tensor.matmul(
                        g_ps[:], xt_blk[:, ts, :, :], W_sb[:],
                        start=(pos == 0), stop=False,
                        perf_mode=mybir.MatmulPerfMode.DoubleRow,
                    )
                    for a in range(KH // 2):
                        nc.tensor.matmul(
                            g_ps[:],
                            hT_prev[:, 2 * a : 2 * a + 2, :],
                            U_sb[:, 2 * a : 2 * a + 2, :],
                            start=False, stop=(a == KH // 2 - 1),
                            perf_mode=mybir.MatmulPerfMode.DoubleRow,
                        )
                else:
                    pos = 0
                    if has_bias:
                        nc.tensor.matmul(g_ps[:], ones_sb[:], b_sb[:],
                                         start=True, stop=False)
                        pos += 1
                    for c in range(KD):
                        nc.tensor.matmul(
                            g_ps[:], xt_blk[:, ts, c, :], W_sb[:, c, :],
                            start=(pos == 0), stop=False,
                        )
                        pos += 1
                    for k in range(KH):
                        nc.tensor.matmul(
                            g_ps[:], hT_prev[:, k, :], U_sb[:, k, :],
                            start=False, stop=(k == KH - 1),
                        )

                # ---- elementwise chain ----
                if strip == "mmonly":
                    continue
                e_all = apool.tile([128, GC], BF16, tag="eall")
                nc.scalar.activation(e_all[:], g_ps[:], AF.Exp,
                                     scale=-1.0 / w_scale)
                if strip != "noln":
                    nc.scalar.activation(spgc[:, 0:HS], e_all[:, 0:HS], AF.Ln,
                                         bias=1.0)
                sig = apool.tile([128, 3 * HS], BF16, tag="sig")
                if strip != "nosig":
                    d_ifo = apool.tile([128, 3 * HS], BF16, tag="difo")
                    nc.vector.tensor_scalar_add(d_ifo[:], e_all[:, HS:GC], 1.0)
                    with nc.allow_low_precision("bf16 sigmoid is fine"):
                        nc.vector.reciprocal(sig[:], d_ifo[:])
                else:
                    nc.vector.tensor_copy(sig[:], e_all[:, HS:GC])

                # [tig | tfc] = [sig_i | sig_f] * [spg | c] in one op
                prodm = apool.tile([128, 2 * HS], F32, tag="prodm")
                nc.vector.tensor_mul(prodm[:], sig[:, 0 : 2 * HS], spgc[:])
                nc.vector.tensor_add(spgc[:, HS : 2 * HS], prodm[:, 0:HS],
                                     prodm[:, HS : 2 * HS])

                h_sb = apool.tile([128, HS], mm_dt, tag="h")
                if strip != "nospc":
                    ec = apool.tile([128, HS], BF16, tag="ec")
                    nc.scalar.activation(ec[:], spgc[:, HS : 2 * HS], AF.Exp)
                    spc = apool.tile([128, HS], F32, tag="spc")
                    nc.scalar.activation(spc[:], ec[:], AF.Ln, bias=1.0)
                    nc.vector.tensor_mul(h_sb[:], sig[:, 2 * HS : 3 * HS], spc[:])
                else:
                    nc.vector.tensor_mul(h_sb[:], sig[:, 2 * HS : 3 * HS],
                                         spgc[:, 0:HS])

                # ---- exchange: AllGather h (untransposed), transposing
                # gather dma assembles hT [h(p), c, b] ----
                if strip == "noex":
                    continue
                if strip == "flatcc":
                    cc_in = dpool.tile([1, B * HS], mm_dt, tag="ccin")
                    nc.sync.dma_start(
                        out=cc_in[:].rearrange("o (b h) -> (o b) h", b=B),
                        in_=h_sb[:])
                    cc_out = dpool.tile([NCORES, B * HS], mm_dt, tag="ccout")
                    nc.gpsimd.collective_compute(
                        "AllGather",
                        mybir.AluOpType.bypass,
                        replica_groups=[list(range(NCORES))],
                        ins=[cc_in[:].opt()],
                        outs=[cc_out[:].opt()],
                    )
                    hT_prev = hT_pool.tile([128, KH, B], mm_dt, tag="hT")
                    nc.sync.dma_start(
                        out=hT_prev[:],
                        in_=cc_out[:].rearrange("c (b h) -> h c b", b=B),
                    )
                    continue
                cc_in = dpool.tile([B, HS], mm_dt, tag="ccin")
                nc.sync.dma_start(out=cc_in[:], in_=h_sb[:])
                if strip == "dmaonly":
                    continue
                cc_out = dpool.tile([NCORES * B, HS], mm_dt, tag="ccout")
                nc.gpsimd.collective_compute(
                    "AllGather",
                    mybir.AluOpType.bypass,
                    replica_groups=[list(range(NCORES))],
                    ins=[cc_in[:].opt()],
                    outs=[cc_out[:].opt()],
                )
                if strip == "nogather":
                    continue
                hT_prev = hT_pool.tile([128, KH, B], mm_dt, tag="hT")
                nc.sync.dma_start(
                    out=hT_prev[:],
                    in_=cc_out[:].rearrange("(c b) h -> h c b", c=NCORES),
                )

            # ---- VAE head ---- (bf16; for fp8 runs hT is fp8 — tolerable
            # only if head_dt == mm_dt, so make a bf16 copy when needed)
            if fp8_dr:
                hT_head = cpool.tile([128, KH, B], head_dt, tag="hTh")
                nc.vector.tensor_copy(hT_head[:], hT_prev[:])
            else:
                hT_head = hT_prev
            mu_ps_t = gps_pool.tile([128, GC], F32, tag="gps")
            lv_ps_t = gps_pool.tile([128, GC], F32, tag="gps")
            mu_ps = mu_ps_t[:, 0:B]
            lv_ps = lv_ps_t[:, 0:B]
            for c in range(KH):
                nc.tensor.matmul(
                    mu_ps, Wm_sb[:, c, :], hT_head[:, c, :],
                    start=(c == 0), stop=(c == KH - 1),
                )
            for c in range(KH):
                nc.tensor.matmul(
                    lv_ps, Wv_sb[:, c, :], hT_head[:, c, :],
                    start=(c == 0), stop=(c == KH - 1),
                )

            mu_sb = apool.tile([Z, B], F32, tag="mu")
            nc.scalar.activation(mu_sb[:], mu_ps, AF.Identity, bias=bm_sb[:])
            lv_sb = apool.tile([Z, B], F32, tag="lv")
            nc.scalar.activation(lv_sb[:], lv_ps, AF.Identity, bias=bv_sb[:])
            e_sb = apool.tile([Z, B], F32, tag="e")
            nc.scalar.activation(
                e_sb[:], lv_ps, AF.Exp, bias=bvh_sb[:], scale=0.5
            )
            ez = apool.tile([Z, B], F32, tag="ez")
            nc.vector.tensor_mul(ez[:], e_sb[:], epsT_sb[:])
            z_sb = apool.tile([Z, B], F32, tag="z")
            nc.vector.tensor_add(z_sb[:], mu_sb[:], ez[:])

            nc.sync.dma_start(out=muT_d.ap(), in_=mu_sb[:])
            nc.sync.dma_start(out=lvT_d.ap(), in_=lv_sb[:])
            nc.sync.dma_start(out=zT_d.ap(), in_=z_sb[:])

    _spill_excess_waits(nc)
    return nc


def make_in_maps(x, W, U, b, Wm, bm, Wv, bv, eps, np_mm_dtype=None,
                 fp8=False, w_scale=1.0):
    import ml_dtypes
    if np_mm_dtype is None:
        np_mm_dtype = ml_dtypes.float8_e4m3 if fp8 else ml_dtypes.bfloat16
    """Host-side pre-processing: transpose x, slice/permute weights per core."""
    T = x.shape[1]
    Tpad = ((T + XBLK - 1) // XBLK) * XBLK
    if Tpad != T:
        x = np.concatenate(
            [x, np.zeros((x.shape[0], Tpad - T, x.shape[2]), x.dtype)], axis=1
        )
    xT = np.ascontiguousarray(x.transpose(1, 2, 0)).astype(np_mm_dtype)  # [T,D,B]
    epsT = np.ascontiguousarray(eps.T).astype(np.float32)  # [Z, B]
    bm_c = np.ascontiguousarray(bm.reshape(Z, 1)).astype(np.float32)
    bv_c = np.ascontiguousarray(bv.reshape(Z, 1)).astype(np.float32)
    bvh_c = np.ascontiguousarray(0.5 * bv.reshape(Z, 1)).astype(np.float32)
    import ml_dtypes as _md
    head_dtype = _md.bfloat16
    Wm_r = np.ascontiguousarray(Wm.reshape(KH, 128, Z)).astype(head_dtype)
    Wv_r = np.ascontiguousarray(Wv.reshape(KH, 128, Z)).astype(head_dtype)

    in_maps = []
    for k in range(NCORES):
        # gate order [g | i | f | o]; Keras kernel order is i,f,g,o
        cols = np.concatenate(
            [
                np.arange(2 * H + k * HS, 2 * H + (k + 1) * HS),  # g
                np.arange(0 * H + k * HS, 0 * H + (k + 1) * HS),  # i
                np.arange(1 * H + k * HS, 1 * H + (k + 1) * HS),  # f
                np.arange(3 * H + k * HS, 3 * H + (k + 1) * HS),  # o
            ]
        )
        # negate the g-block columns so one Exp(scale=-1) serves all gates
        neg = np.ones((GC,), np.float32)
        neg[:HS] = -1.0
        negs = neg * w_scale
        Usl = np.ascontiguousarray(U[:, cols] * negs).reshape(KH, 128, GC).astype(
            np_mm_dtype
        )
        Wsl = np.ascontiguousarray(W[:, cols] * negs).reshape(KD, 128, GC).astype(
            np_mm_dtype
        )
        bsl = (b[cols] * negs).reshape(1, GC).astype(np_mm_dtype)
        in_maps.append(
            {
                "xT": xT,
                "Usl": Usl,
                "Wsl": Wsl,
                "bsl": bsl,
                "Wm": Wm_r,
                "Wv": Wv_r,
                "bm": bm_c,
                "bv": bv_c,
                "bvh": bvh_c,
                "epsT": epsT,
            }
        )
    return in_maps


def postprocess(core0_out):
    mu = np.ascontiguousarray(core0_out["muT"].T).astype(np.float32)
    logvar = np.ascontiguousarray(core0_out["logvarT"].T).astype(np.float32)
    z = np.ascontiguousarray(core0_out["zT"].T).astype(np.float32)
    return mu, logvar, z


# ----------------------------------------------------------------------------
# Harness entry point: full (unsharded) inputs -> full outputs.
# ----------------------------------------------------------------------------
USE_FP8 = False
_NC_CACHE = {}


def kernel(x, W, U, b, Wm, bm, Wv, bv, eps):
    import time as _time

    from concourse.bass_utils import run_bass_kernel_spmd

    x = np.asarray(x, dtype=np.float32)
    W = np.asarray(W, dtype=np.float32)
    U = np.asarray(U, dtype=np.float32)
    b = np.asarray(b, dtype=np.float32)
    Wm = np.asarray(Wm, dtype=np.float32)
    bm = np.asarray(bm, dtype=np.float32)
    Wv = np.asarray(Wv, dtype=np.float32)
    bv = np.asarray(bv, dtype=np.float32)
    eps = np.asarray(eps, dtype=np.float32)

    T = x.shape[1]
    has_bias = bool(np.any(b != 0.0))
    key = (T, has_bias, USE_FP8)
    if key not in _NC_CACHE:
        _NC_CACHE[key] = build_nc(T, has_bias=has_bias, fp8_dr=USE_FP8)
    nc = _NC_CACHE[key]
    in_maps = make_in_maps(x, W, U, b, Wm, bm, Wv, bv, eps, fp8=USE_FP8)
    last = None
    for _attempt in range(3):
        try:
            res = run_bass_kernel_spmd(nc, in_maps, core_ids=list(range(NCORES)))
            return postprocess(res.results[0])
        except Exception as e:  # transient device hiccups: retry
            last = e
            _time.sleep(2.0)
    raise last
